# revision 14
# baseline (speedup 1.0000x reference)
"""Trainium2 Bass kernel for a pre-LN transformer block (MHA + MLP).

Strategy (v3):
  - Data-parallel over batch: 32 batches -> 4 per core x 8 cores.
  - Transposed layout [C, T] on device; host transposes in/out.
  - LayerNorm folding (fast path, all biases zero): the centering
    (I - 11^T/C) and gamma are folded into the projection weights on the
    host, so Q/K/V/MLP1 matmuls consume RAW x. The per-token
    alpha = rsqrt(var+eps) is applied:
      * to q~/k~ during PSUM evacuation (tensor_tensor mult = free),
      * to v~ via tensor_scalar with an alpha COLUMN (tokens are
        partitions there); the column comes from a PE transpose of the
        partition-replicated alpha tile,
      * for the MLP, relu's positive homogeneity commutes alpha2 to the
        output: y = x1 + alpha2 * (relu(x1@W1C) @ W2).
    This removes the entire elementwise LN apply.
  - alpha = exp(-0.5*ln(var+eps)) on ScalarE: Ln/Exp share one
    activation table set with the attention exp (scalar Rsqrt is banned,
    DVE reciprocal costs 8 cyc/elem, custom DVE ops don't compile).
  - Attention: head pairs at partition offsets 0/64 issue score matmuls
    to disjoint PE row groups (concurrent in the array); both heads'
    scores land in one [128, 2, 512] PSUM tile (one bank per head) and
    a single Exp covers the pair. Causal mask = extra matmul
    accumulating -1e30 into the diagonal block; exp underflows to 0.
    Softmax denominators ride as 64 ones-columns in the PV stationary;
    1/l = exp(-ln(l)) on ScalarE.
  - Per-batch phase interleave ATTN->WO->LN2->MLP keeps the PE fed
    (MLP matmuls depend only on x1) while the next batch's attention
    waits on ScalarE exps; keeps the HAM clock warm.
  - Residuals updated in-place in the xt tiles (f32 accuracy kept).
"""

import numpy as np
import ml_dtypes

import concourse.bass as bass
import concourse.mybir as mybir
import concourse.tile as tile
from concourse.bass_utils import run_bass_kernel_spmd

# ---- problem constants (hardcoded per harness contract) ----
B = 32
T = 768
C = 256
H = 4
HS = 64  # head size
F = 4 * C  # 1024
N_CORES = 8
B_PER_CORE = B // N_CORES  # 4
LN_EPS = 1e-5
F32 = mybir.dt.float32
F32R = mybir.dt.float32r
BF16 = mybir.dt.bfloat16

AF = mybir.ActivationFunctionType
ALU = mybir.AluOpType

NEG_BIG = -1e30


def chunks(lo, hi, cap):
    """Greedy split of [lo, hi) into pieces of at most cap."""
    out = []
    while lo < hi:
        ln = min(cap, hi - lo)
        out.append((lo, ln))
        lo += ln
    return out


# Attention query-chunk units per key-block si: (si, qlo, width).
# Width cap 512: the [128, 2, 512] pair score tile is exactly 2 PSUM
# banks, one bank per head, so each matmul output stays within a bank.
ATTN_UNITS = []
for _si in range(6):
    for _qlo, _w in chunks(_si * 128, T, 512):
        ATTN_UNITS.append((_si, _qlo, _w))


# This walrus build rejects >1 sem wait per instruction (setupSyncWait
# "Too many sync wait commands"). Post-pass: move excess waits onto
# freshly inserted same-engine NoOps immediately before the offender.
_MAX_WAITS = 1


def _split_waits(nc):
    n_new = 0
    for bass_bb in nc.bb_map.values():
        bb = bass_bb.bb
        insts = list(bb.instructions)
        out = []
        changed = False
        for inst in insts:
            si = getattr(inst, "sync_info", None)
            waits = list(si.on_wait) if si and si.on_wait else []
            if len(waits) > _MAX_WAITS:
                changed = True
                excess, keep = waits[:-_MAX_WAITS], waits[-_MAX_WAITS:]
                for j in range(0, len(excess), _MAX_WAITS):
                    nop = mybir.InstNoOp(name=f"waitnop-{n_new}", ins=[], outs=[])
                    n_new += 1
                    nop.engine = inst.engine
                    nop.sync_info = mybir.SyncInfo(
                        on_wait=excess[j:j + _MAX_WAITS], on_update=[])
                    out.append(nop)
                inst.sync_info = mybir.SyncInfo(
                    on_wait=keep, on_update=list(si.on_update))
            out.append(inst)
        if changed:
            bb.instructions = out
    return n_new


def _build_nc(has_qkvb, has_bo, has_mlpb):
    """has_qkvb: nonzero folded q/k/v biases (be1 != 0) -> general LN1.
    has_bo: nonzero bo. has_mlpb: nonzero b1/b2/be2 -> general LN2/MLP."""
    nc = bass.Bass("TRN2", target_bir_lowering=False, debug=False,
                   num_devices=N_CORES)
    fast1 = not has_qkvb
    fast2 = not has_mlpb

    P = nc.declare_dram_parameter
    xt_d = P("xt", [B_PER_CORE, C, T], F32R, isOutput=False)
    pdt = F32R if fast1 else BF16
    wq_d = P("wq", [2, 128, C], pdt, isOutput=False)
    wk_d = P("wk", [2, 128, C], pdt, isOutput=False)
    wv_d = P("wv", [2, 128, C], pdt, isOutput=False)
    wo_d = P("wo", [2, 128, C], BF16, isOutput=False)
    w1_d = P("w1", [2, 128, F], F32R if fast2 else BF16, isOutput=False)
    w2_d = P("w2", [8, 128, C], BF16, isOutput=False)
    maskt_d = P("maskt", [128, 128], BF16, isOutput=False)
    ident_d = P("ident", [128, 128], BF16, isOutput=False)
    id32_d = P("id32", [128, 128], F32, isOutput=False)
    onc_d = P("ones_c", [128, 128], F32R, isOutput=False)
    onv_d = P("ones_va", [128, C], BF16, isOutput=False)
    ont_d = P("ones_t", [1, T], BF16, isOutput=False)
    bq_d = P("bq", [128, 2], F32, isOutput=False)
    bk_d = P("bk", [128, 2], F32, isOutput=False)
    bv_d = P("bv", [128, C], F32, isOutput=False)
    bo_d = P("bo", [1, C], BF16, isOutput=False)
    b1_d = P("b1", [128, 8], F32, isOutput=False)
    b2_d = P("b2", [1, C], BF16, isOutput=False)
    yt_d = P("yt", [B_PER_CORE, C, T], F32, isOutput=True)

    with tile.TileContext(nc) as tc:
        with (
            tc.tile_pool(name="consts", bufs=1) as consts,
            tc.tile_pool(name="per_b", bufs=1) as per_b,
            tc.tile_pool(name="work", bufs=2) as work,
            tc.tile_pool(name="psum", bufs=2, space="PSUM") as psum,
        ):
            _kernel_body(
                nc, consts, per_b, work, psum,
                xt_d, wq_d, wk_d, wv_d, wo_d, w1_d, w2_d,
                maskt_d, ident_d, id32_d, onc_d, onv_d, ont_d,
                bq_d, bk_d, bv_d, bo_d, b1_d, b2_d, yt_d,
                fast1, fast2, has_qkvb, has_bo, has_mlpb,
            )
    _split_waits(nc)
    return nc


def _kernel_body(nc, consts, per_b, work, psum,
                 xt_d, wq_d, wk_d, wv_d, wo_d, w1_d, w2_d,
                 maskt_d, ident_d, id32_d, onc_d, onv_d, ont_d,
                 bq_d, bk_d, bv_d, bo_d, b1_d, b2_d, yt_d,
                 fast1, fast2, has_qkvb, has_bo, has_mlpb):
    NB = B_PER_CORE
    pdt = F32R if fast1 else BF16
    mdt = F32R if fast2 else BF16

    # ---- warm the ACT natural_log_exp table set while DMAs run ----
    eps_sb = consts.tile([128, 1], F32, tag="eps")
    nc.vector.memset(eps_sb, LN_EPS)
    warm_sb = consts.tile([128, 1], F32, tag="actwarm")
    nc.scalar.activation(out=warm_sb, in_=eps_sb, func=AF.Exp)
    nc.scalar.activation(out=warm_sb, in_=eps_sb, func=AF.Ln, bias=eps_sb,
                         scale=1.0)

    # ---- x first: everything downstream waits on it ----
    ones_stat = consts.tile([128, 128], F32R, tag="ones_stat")
    nc.sync.dma_start(out=ones_stat, in_=onc_d[:, :])
    xt = [[per_b.tile([128, T], F32R, tag=f"xt{b}_{ct}", name=f"xt{b}_{ct}")
           for ct in range(2)] for b in range(NB)]
    for b in range(NB):
        for ct in range(2):
            nc.sync.dma_start(out=xt[b][ct],
                              in_=xt_d[b, ct * 128:(ct + 1) * 128, :])

    # ---- constants ----
    wq_sb = [consts.tile([128, C], pdt, tag=f"wq{i}", name=f"wq{i}")
             for i in range(2)]
    wk_sb = [consts.tile([128, C], pdt, tag=f"wk{i}", name=f"wk{i}")
             for i in range(2)]
    wv_sb = [consts.tile([128, C], pdt, tag=f"wv{i}", name=f"wv{i}")
             for i in range(2)]
    wo_sb = [consts.tile([128, C], BF16, tag=f"wo{i}", name=f"wo{i}")
             for i in range(2)]
    w1_sb = [consts.tile([128, F], mdt, tag=f"w1{i}", name=f"w1{i}")
             for i in range(2)]
    w2_sb = [consts.tile([128, C], BF16, tag=f"w2{i}", name=f"w2{i}")
             for i in range(8)]
    for kt in range(2):
        nc.sync.dma_start(out=wq_sb[kt], in_=wq_d[kt])
        nc.sync.dma_start(out=wk_sb[kt], in_=wk_d[kt])
        nc.sync.dma_start(out=wv_sb[kt], in_=wv_d[kt])
        nc.sync.dma_start(out=wo_sb[kt], in_=wo_d[kt])
        nc.sync.dma_start(out=w1_sb[kt], in_=w1_d[kt])
    for kt in range(8):
        nc.sync.dma_start(out=w2_sb[kt], in_=w2_d[kt])

    maskt_sb = consts.tile([128, 128], BF16, tag="maskt")
    ident_sb = consts.tile([128, 128], BF16, tag="ident")
    ones_va = consts.tile([128, C], BF16, tag="ones_va")
    nc.sync.dma_start(out=maskt_sb, in_=maskt_d[:, :])
    nc.sync.dma_start(out=ident_sb, in_=ident_d[:, :])
    nc.sync.dma_start(out=ones_va, in_=onv_d[:, :])
    if fast1:
        id32_sb = consts.tile([128, 128], F32, tag="id32")
        nc.sync.dma_start(out=id32_sb, in_=id32_d[:, :])
    if has_qkvb:
        bq_sb = consts.tile([128, 2], F32, tag="bq")
        bk_sb = consts.tile([128, 2], F32, tag="bk")
        bv_sb = consts.tile([128, C], F32, tag="bv")
        nc.sync.dma_start(out=bq_sb, in_=bq_d[:, :])
        nc.sync.dma_start(out=bk_sb, in_=bk_d[:, :])
        nc.sync.dma_start(out=bv_sb, in_=bv_d[:, :])
    if has_mlpb:
        b1_sb = consts.tile([128, 8], F32, tag="b1")
        nc.sync.dma_start(out=b1_sb, in_=b1_d[:, :])
    if has_bo or has_mlpb:
        ones_row = consts.tile([1, T], BF16, tag="ones_row")
        nc.sync.dma_start(out=ones_row, in_=ont_d[:, :])
        bo_sb = consts.tile([1, C], BF16, tag="bo")
        nc.sync.dma_start(out=bo_sb, in_=bo_d[:, :])
        b2_sb = consts.tile([1, C], BF16, tag="b2")
        nc.sync.dma_start(out=b2_sb, in_=b2_d[:, :])

    # ---- per-batch persistent tiles ----
    q_sb = [[per_b.tile([128, T], BF16, tag=f"q{b}_{mt}", name=f"q{b}_{mt}")
             for mt in range(2)] for b in range(NB)]
    k_sb = [[per_b.tile([128, T], BF16, tag=f"k{b}_{mt}", name=f"k{b}_{mt}")
             for mt in range(2)] for b in range(NB)]
    vaug = [[per_b.tile([128, H, 128], BF16, tag=f"va{b}_{tt}",
                        name=f"va{b}_{tt}")
             for tt in range(6)] for b in range(NB)]
    ot = [[per_b.tile([128, T], BF16, tag=f"ot{b}_{mt}", name=f"ot{b}_{mt}")
           for mt in range(2)] for b in range(NB)]
    if not (fast1 and fast2):
        ht = [[per_b.tile([128, T], BF16, tag=f"ht{b}_{ct}",
                          name=f"ht{b}_{ct}") for ct in range(2)]
              for b in range(NB)]

    # ones halves of vaug: written once per run
    for b in range(NB):
        for tt in range(6):
            nc.vector.tensor_copy(
                out=vaug[b][tt][:, :, 64:128],
                in_=ones_va.rearrange("p (h d) -> p h d", h=H))

    def ln_stats(b, src, tag):
        """Partition-replicated LN stats for src (2x [128,T] f32r).
        Returns (alpha [128,T] f32 SBUF, ps_mu PSUM)."""
        sq = [work.tile([128, T], F32R, tag=f"ln_sq{ct}", bufs=2,
                        name=f"{tag}_sq{ct}") for ct in range(2)]
        for ct in range(2):
            nc.gpsimd.tensor_tensor(out=sq[ct], in0=src[ct], in1=src[ct],
                                    op=ALU.mult)
        ps_mu = psum.tile([128, T], F32, tag="pA", name=f"{tag}_mu")
        ps_ex2 = psum.tile([128, T], F32, tag="pB", name=f"{tag}_ex2")
        for ps, rhs in ((ps_mu, src), (ps_ex2, sq)):
            for kt in range(2):
                for st, ln in chunks(0, T, 512):
                    nc.tensor.matmul(
                        ps[:, st:st + ln], ones_stat, rhs[kt][:, st:st + ln],
                        start=(kt == 0), stop=(kt == 1))
        t2 = work.tile([128, T], F32, tag="ln_t2", bufs=2, name=f"{tag}_t2")
        alpha = work.tile([128, T], F32, tag="ln_al", bufs=4,
                          name=f"{tag}_al")
        nc.scalar.activation(out=t2, in_=ps_mu, func=AF.Square)
        nc.vector.tensor_tensor(out=t2, in0=ps_ex2, in1=t2, op=ALU.subtract)
        # alpha = (var+eps)^-0.5 = exp(-0.5*ln(var+eps)); Ln/Exp share the
        # natural_log_exp table set with the attention exp (no table swap).
        nc.scalar.activation(out=t2, in_=t2, func=AF.Ln, bias=eps_sb,
                             scale=1.0)
        nc.scalar.activation(out=alpha, in_=t2, func=AF.Exp, scale=-0.5)
        return alpha, ps_mu

    def ln_apply(b, src, alpha, ps_mu, out_tiles, tag):
        """General path: materialize h = (x - mu) * alpha into out_tiles."""
        beta = work.tile([128, T], F32, tag="ln_be", bufs=2,
                         name=f"{tag}_be")
        nc.vector.tensor_tensor(out=beta, in0=ps_mu, in1=alpha, op=ALU.mult)
        for ct in range(2):
            g1 = work.tile([128, T], F32, tag=f"ln_g{ct}", bufs=2,
                           name=f"{tag}_g{ct}")
            nc.gpsimd.tensor_tensor(out=g1, in0=src[ct], in1=alpha,
                                    op=ALU.mult)
            nc.gpsimd.tensor_tensor(out=out_tiles[ct], in0=g1, in1=beta,
                                    op=ALU.subtract)

    # ================= LN1 =================
    alphas = {}
    acol = {}
    for b in range(NB):
        alpha, ps_mu = ln_stats(b, xt[b], f"ln1_{b}")
        alphas[b] = alpha
        if not fast1:
            ln_apply(b, xt[b], alpha, ps_mu, ht[b], f"ln1_{b}")

    # ================= QKV =================
    for b in range(NB):
        alpha = alphas[b]
        mov = xt[b] if fast1 else ht[b]
        for name, w_sb, dst, bias_sb in (
                ("q", wq_sb, q_sb[b], bq_sb if has_qkvb else None),
                ("k", wk_sb, k_sb[b], bk_sb if has_qkvb else None)):
            for mt in range(2):
                ps = psum.tile([128, T], F32, tag="pA" if mt == 0 else "pB",
                               name=f"ps_{name}{b}_{mt}")
                for kt in range(2):
                    for st, ln in chunks(0, T, 512):
                        nc.tensor.matmul(
                            ps[:, st:st + ln],
                            w_sb[kt][:, mt * 128:(mt + 1) * 128],
                            mov[kt][:, st:st + ln],
                            start=(kt == 0), stop=(kt == 1))
                if fast1:
                    nc.vector.tensor_tensor(out=dst[mt], in0=ps, in1=alpha,
                                            op=ALU.mult)
                elif has_qkvb:
                    nc.scalar.activation(out=dst[mt], in_=ps,
                                         func=AF.Identity,
                                         bias=bias_sb[:, mt:mt + 1],
                                         scale=1.0)
                else:
                    nc.vector.tensor_copy(out=dst[mt], in_=ps)
        if fast1:
            # alpha columns (tokens on partitions) via PE transpose of the
            # replicated alpha tile; column 0 of each transposed block.
            ps_t = psum.tile([128, 6, 128], F32, tag="pA", name=f"ps_t{b}")
            for s in range(6):
                nc.tensor.transpose(ps_t[:, s, :],
                                    alpha[:, s * 128:(s + 1) * 128], id32_sb)
            ac = work.tile([128, 6], F32, tag="acol", bufs=4,
                           name=f"acol{b}")
            nc.vector.tensor_copy(out=ac, in_=ps_t[:, :, 0:1])
            acol[b] = ac
        for tt in range(6):
            ps = psum.tile([128, C], F32, tag="pA" if tt % 2 == 0 else "pB",
                           name=f"ps_v{b}_{tt}")
            for kt in range(2):
                nc.tensor.matmul(
                    ps, mov[kt][:, tt * 128:(tt + 1) * 128], wv_sb[kt],
                    start=(kt == 0), stop=(kt == 1))
            if fast1:
                nc.vector.tensor_scalar(
                    out=vaug[b][tt][:, :, 0:64],
                    in0=ps.rearrange("p (h d) -> p h d", h=H),
                    scalar1=acol[b][:, tt:tt + 1], scalar2=None,
                    op0=ALU.mult)
            elif has_qkvb:
                nc.vector.tensor_tensor(
                    out=vaug[b][tt][:, :, 0:64],
                    in0=ps.rearrange("p (h d) -> p h d", h=H),
                    in1=bv_sb.rearrange("p (h d) -> p h d", h=H),
                    op=ALU.add)
            else:
                nc.vector.tensor_copy(
                    out=vaug[b][tt][:, :, 0:64],
                    in_=ps.rearrange("p (h d) -> p h d", h=H))

    # ============ pipelined: ATTN / WO / (LN2+MLP) blocks ============
    def attn_block(b):
        # ---- attention ----
        for mt in range(2):
            po = [psum.tile([128, T], F32, tag="pB", name=f"po{b}_{mt}_{hh}")
                  for hh in range(2)]

            def emit_pv(unit):
                usi, uqlo, uw, upt = unit
                for hh in range(2):
                    nc.tensor.matmul(
                        po[hh][:, uqlo:uqlo + uw],
                        vaug[b][usi][:, 2 * mt + hh, :],
                        upt[:, hh, 0:uw],
                        start=(usi == 0), stop=(usi == 5))

            # PV trails scores/exp by one unit so the in-order PE stream
            # never waits on a fresh exp. Each unit gets its OWN pt tile:
            # deferring with a shared pt would read columns the next
            # unit's exp already overwrote.
            pending = None
            for si, qlo, w in ATTN_UNITS:
                diag = (qlo == si * 128)
                ps_s = psum.tile([128, 2, 512], F32, tag="pA",
                                 name=f"ps_s{b}_{mt}_{si}_{qlo}")
                for hh in range(2):
                    nc.tensor.matmul(
                        ps_s[:, hh, 0:w],
                        k_sb[b][mt][hh * 64:hh * 64 + 64,
                                    si * 128:si * 128 + 128],
                        q_sb[b][mt][hh * 64:hh * 64 + 64, qlo:qlo + w],
                        start=True, stop=not diag)
                if diag:
                    for hh in range(2):
                        nc.tensor.matmul(
                            ps_s[:, hh, 0:128], maskt_sb, ident_sb,
                            start=False, stop=True)
                pt = work.tile([128, 2, 512], BF16, tag="ptp", bufs=3,
                               name=f"pt{b}_{mt}_{si}_{qlo}")
                nc.scalar.activation(out=pt[:, :, 0:w],
                                     in_=ps_s[:, :, 0:w],
                                     func=AF.Exp, scale=HS ** -0.5)
                if pending is not None:
                    emit_pv(pending)
                pending = (si, qlo, w, pt)
            emit_pv(pending)
            rb = work.tile([64, 2, T], F32, tag="rb", bufs=2,
                           name=f"rb{b}_{mt}")
            for hh in range(2):
                nc.scalar.activation(out=rb[:, hh, :], in_=po[hh][64:128, :],
                                     func=AF.Ln)
            nc.scalar.activation(out=rb, in_=rb, func=AF.Exp, scale=-1.0)
            for hh in range(2):
                nc.vector.tensor_tensor(
                    out=ot[b][mt][hh * 64:hh * 64 + 64, :],
                    in0=po[hh][0:64, :], in1=rb[:, hh, :], op=ALU.mult)

    def wo_block(b):
        # ---- Wo + residual (in-place into xt) ----
        for mt in range(2):
            ps = psum.tile([128, T], F32, tag="pA", name=f"ps_r{b}_{mt}")
            for kt in range(2):
                for st, ln in chunks(0, T, 512):
                    nc.tensor.matmul(
                        ps[:, st:st + ln],
                        wo_sb[kt][:, mt * 128:(mt + 1) * 128],
                        ot[b][kt][:, st:st + ln],
                        start=(kt == 0), stop=(kt == 1) and not has_bo)
            if has_bo:
                for st, ln in chunks(0, T, 512):
                    nc.tensor.matmul(
                        ps[:, st:st + ln],
                        bo_sb[0:1, mt * 128:(mt + 1) * 128],
                        ones_row[:, st:st + ln], start=False, stop=True)
            nc.vector.tensor_tensor(out=xt[b][mt], in0=ps, in1=xt[b][mt],
                                    op=ALU.add)

    def post_block(b):
        # ---- LN2 ----
        alpha2, ps_mu2 = ln_stats(b, xt[b], f"ln2_{b}")
        if not fast2:
            ln_apply(b, xt[b], alpha2, ps_mu2, ht[b], f"ln2_{b}")

        # ---- MLP ----
        mov2 = xt[b] if fast2 else ht[b]
        ps_y = [psum.tile([128, T], F32, tag="pB", name=f"ps_y{b}_{mt}")
                for mt in range(2)]

        def emit_mlp2(f, ut):
            for mt in range(2):
                for st, ln in chunks(0, T, 512):
                    nc.tensor.matmul(
                        ps_y[mt][:, st:st + ln],
                        w2_sb[f][:, mt * 128:(mt + 1) * 128],
                        ut[:, st:st + ln],
                        start=(f == 0), stop=(f == 7) and not has_mlpb)

        pend = None  # MLP2 trails MLP1/relu by one f-block
        for f in range(8):
            ps_u = psum.tile([128, T], F32, tag="pA", name=f"ps_u{b}_{f}")
            for kt in range(2):
                for st, ln in chunks(0, T, 512):
                    nc.tensor.matmul(
                        ps_u[:, st:st + ln],
                        w1_sb[kt][:, f * 128:(f + 1) * 128],
                        mov2[kt][:, st:st + ln],
                        start=(kt == 0), stop=(kt == 1))
            ut = work.tile([128, T], BF16, tag="ut", bufs=4,
                           name=f"ut{b}_{f}")
            if has_mlpb:
                nc.scalar.activation(out=ut, in_=ps_u, func=AF.Relu,
                                     bias=b1_sb[:, f:f + 1], scale=1.0)
            elif f % 4 == 0:
                nc.scalar.activation(out=ut, in_=ps_u, func=AF.Relu)
            else:
                nc.vector.tensor_scalar_max(out=ut, in0=ps_u, scalar1=0.0)
            if pend is not None:
                emit_mlp2(*pend)
            pend = (f, ut)
        emit_mlp2(*pend)
        for mt in range(2):
            if has_mlpb:
                for st, ln in chunks(0, T, 512):
                    nc.tensor.matmul(
                        ps_y[mt][:, st:st + ln],
                        b2_sb[0:1, mt * 128:(mt + 1) * 128],
                        ones_row[:, st:st + ln], start=False, stop=True)
            if fast2:
                my = work.tile([128, T], F32, tag="my", bufs=2,
                               name=f"my{b}_{mt}")
                nc.vector.tensor_tensor(out=my, in0=ps_y[mt], in1=alpha2,
                                        op=ALU.mult)
                nc.vector.tensor_tensor(out=xt[b][mt], in0=my,
                                        in1=xt[b][mt], op=ALU.add)
            else:
                nc.vector.tensor_tensor(out=xt[b][mt], in0=ps_y[mt],
                                        in1=xt[b][mt], op=ALU.add)
            nc.sync.dma_start(out=yt_d[b, mt * 128:(mt + 1) * 128, :],
                              in_=xt[b][mt].bitcast(F32))

    # Emission order: an independent ATTN block sits between each WO and
    # its dependent LN2/MLP so the serial residual-add -> square -> stats
    # chain never starves the in-order PE stream.
    attn_block(0)
    attn_block(1)
    wo_block(0)
    attn_block(2)
    post_block(0)
    wo_block(1)
    attn_block(3)
    post_block(1)
    wo_block(2)
    wo_block(3)
    post_block(2)
    post_block(3)


_NC_CACHE = {}


def _prep_weights(Wq, Wk, Wv, Wo, bo, W1, b1, W2, b2, g1, be1, g2, be2,
                  fast1, fast2):
    f64 = np.float64
    g1, be1 = g1.astype(f64), be1.astype(f64)
    g2, be2 = g2.astype(f64), be2.astype(f64)
    CEN = np.eye(C) - np.ones((C, C)) / C  # centering projector

    def fold_qkv(W):  # W: [H, C, HS] -> folded [C, H*HS], bias [H*HS]
        Wraw = np.concatenate([W[h].astype(f64) for h in range(H)], axis=1)
        Wg = Wraw * g1[:, None]
        bias = be1 @ Wraw
        if fast1:
            Wg = CEN @ Wg
        return Wg, bias

    WqF, bq = fold_qkv(Wq)
    WkF, bk = fold_qkv(Wk)
    WvF, bv = fold_qkv(Wv)
    W1F = W1.astype(f64) * g2[:, None]
    b1F = b1.astype(f64) + be2 @ W1.astype(f64)
    if fast2:
        W1F = CEN @ W1F

    def f32(a):
        return np.ascontiguousarray(a, dtype=np.float32)

    def bf16(a):
        return np.ascontiguousarray(
            np.asarray(a, f64).astype(ml_dtypes.bfloat16))

    pcast = f32 if fast1 else bf16
    mcast = f32 if fast2 else bf16
    r = np.arange(128)
    maskt = np.where(r[None, :] <= r[:, None], 0.0, NEG_BIG)  # [k, m]

    return {
        "wq": pcast(WqF.reshape(2, 128, C)),
        "wk": pcast(WkF.reshape(2, 128, C)),
        "wv": pcast(WvF.reshape(2, 128, C)),
        "wo": bf16(np.asarray(Wo, f64).reshape(2, 128, C)),
        "w1": mcast(W1F.reshape(2, 128, F)),
        "w2": bf16(np.asarray(W2, f64).reshape(8, 128, C)),
        "bq": f32(bq.reshape(2, 128).T),
        "bk": f32(bk.reshape(2, 128).T),
        "bv": f32(np.broadcast_to(bv, (128, C))),
        "bo": bf16(np.asarray(bo, f64).reshape(1, C)),
        "b1": f32(b1F.reshape(8, 128).T),
        "b2": bf16(np.asarray(b2, f64).reshape(1, C)),
        "maskt": bf16(maskt),
        "ident": bf16(np.eye(128)),
        "id32": f32(np.eye(128)),
        "ones_c": f32(np.full((128, 128), 1.0 / C)),
        "ones_va": bf16(np.ones((128, C))),
        "ones_t": bf16(np.ones((1, T))),
    }


def kernel(x, Wq, Wk, Wv, Wo, bo, W1, b1, W2, b2, g1, be1, g2, be2,
           _trace=False):
    x = np.asarray(x, dtype=np.float32)
    has_qkvb = bool(np.any(np.asarray(be1)))
    has_bo = bool(np.any(np.asarray(bo)))
    has_mlpb = bool(np.any(np.asarray(b1)) or np.any(np.asarray(b2))
                    or np.any(np.asarray(be2)))
    flags = (has_qkvb, has_bo, has_mlpb)

    weights = _prep_weights(
        np.asarray(Wq), np.asarray(Wk), np.asarray(Wv), np.asarray(Wo),
        np.asarray(bo), np.asarray(W1), np.asarray(b1), np.asarray(W2),
        np.asarray(b2), np.asarray(g1), np.asarray(be1), np.asarray(g2),
        np.asarray(be2), not has_qkvb, not has_mlpb)

    if flags not in _NC_CACHE:
        _NC_CACHE[flags] = _build_nc(*flags)
    nc = _NC_CACHE[flags]

    xt = np.ascontiguousarray(x.transpose(0, 2, 1))  # [B, C, T]
    in_maps = []
    for core in range(N_CORES):
        m = dict(weights)
        m["xt"] = np.ascontiguousarray(
            xt[core * B_PER_CORE:(core + 1) * B_PER_CORE])
        in_maps.append(m)

    res = run_bass_kernel_spmd(nc, in_maps, list(range(N_CORES)),
                               trace=_trace)
    outs = [res.results[i]["yt"] for i in range(N_CORES)]  # [4, C, T] each
    y = np.concatenate(outs, axis=0).transpose(0, 2, 1)  # [B, T, C]
    if _trace:
        kernel.last_exec_time_ns = res.exec_time_ns
        kernel.last_results = res
    return np.ascontiguousarray(y)


# revision 17
# speedup vs baseline: 1.0282x; 1.0282x over previous
"""Trainium2 Bass kernel for a pre-LN transformer block (MHA + MLP).

Design (final):
  - Data-parallel over batch: 32 batches -> 4 per core x 8 cores; full
    inputs sharded on host, outputs gathered on host.
  - Transposed layout [C, T] on device; host transposes in/out.
  - LayerNorm folding (fast path, all biases zero): centering
    (I - 11^T/C) and gamma fold into the projection weights on the host,
    so Q/K/V/MLP1 matmuls consume RAW x (f32r). The per-token
    alpha = rsqrt(var+eps) is applied:
      * to q~/k~ during PSUM evacuation (tensor_tensor mult = free),
      * to v~ via tensor_scalar with an alpha COLUMN (tokens are
        partitions there); the column comes from a PE transpose of the
        partition-replicated alpha tile,
      * for the MLP, relu's positive homogeneity commutes alpha2 to the
        output: y = x1 + alpha2 * (relu(x1@W1C) @ W2).
    This removes the entire elementwise LN apply. A general (nonzero
    bias) fallback path materializes h the classic way.
  - All reciprocals via exp(-ln(x)) on ScalarE: Ln/Exp share one
    activation table set with the attention exp, so the whole kernel
    does a single ACT table load (warmed at t=0 under the input DMAs).
    (Scalar Rsqrt/Reciprocal are banned in this bass; DVE reciprocal is
    8 cyc/elem; custom DVE ops fail this walrus codegen.)
  - Attention: head pairs at partition offsets 0/64 issue score matmuls
    to disjoint PE row groups (concurrent in the array); both heads'
    scores land in one [128, 2, 512] PSUM tile (one bank per head) and
    a single Exp covers the pair. Causal mask = an extra matmul that
    accumulates -1e30 into the diagonal block (maskT x identity);
    exp underflows to exact 0. Softmax denominators ride as 64
    ones-columns in the PV stationary.
  - Software pipelining everywhere: PV matmuls trail scores/exp by one
    unit (each unit owns its pt tile - a shared pt would be overwritten
    before the deferred read), MLP2 trails MLP1/relu by one f-block, and
    blocks interleave as A0 A1 WO0 A2 P0 WO1 A3 P1 WO2 WO3 P2 P3 so the
    in-order PE stream always has independent work while residual-add ->
    stats chains resolve. Residuals update in-place in the xt tiles.
"""

import numpy as np
import ml_dtypes

import concourse.bass as bass
import concourse.mybir as mybir
import concourse.tile as tile
from concourse.bass_utils import run_bass_kernel_spmd

# ---- problem constants (hardcoded per harness contract) ----
B = 32
T = 768
C = 256
H = 4
HS = 64  # head size
F = 4 * C  # 1024
N_CORES = 8
B_PER_CORE = B // N_CORES  # 4
LN_EPS = 1e-5
F32 = mybir.dt.float32
F32R = mybir.dt.float32r
BF16 = mybir.dt.bfloat16

AF = mybir.ActivationFunctionType
ALU = mybir.AluOpType

NEG_BIG = -1e30


def chunks(lo, hi, cap):
    """Greedy split of [lo, hi) into pieces of at most cap."""
    out = []
    while lo < hi:
        ln = min(cap, hi - lo)
        out.append((lo, ln))
        lo += ln
    return out


# Attention query-chunk units per key-block si: (si, qlo, width).
# Width cap 512: the [128, 2, 512] pair score tile is exactly 2 PSUM
# banks, one bank per head, so each matmul output stays within a bank.
ATTN_UNITS = []
for _si in range(6):
    for _qlo, _w in chunks(_si * 128, T, 512):
        ATTN_UNITS.append((_si, _qlo, _w))


# This walrus build rejects >1 sem wait per instruction (setupSyncWait
# "Too many sync wait commands"). Post-pass: move excess waits onto
# freshly inserted same-engine NoOps immediately before the offender.
_MAX_WAITS = 1


def _split_waits(nc):
    n_new = 0
    for bass_bb in nc.bb_map.values():
        bb = bass_bb.bb
        insts = list(bb.instructions)
        out = []
        changed = False
        for inst in insts:
            si = getattr(inst, "sync_info", None)
            waits = list(si.on_wait) if si and si.on_wait else []
            if len(waits) > _MAX_WAITS:
                changed = True
                excess, keep = waits[:-_MAX_WAITS], waits[-_MAX_WAITS:]
                for j in range(0, len(excess), _MAX_WAITS):
                    nop = mybir.InstNoOp(name=f"waitnop-{n_new}", ins=[], outs=[])
                    n_new += 1
                    nop.engine = inst.engine
                    nop.sync_info = mybir.SyncInfo(
                        on_wait=excess[j:j + _MAX_WAITS], on_update=[])
                    out.append(nop)
                inst.sync_info = mybir.SyncInfo(
                    on_wait=keep, on_update=list(si.on_update))
            out.append(inst)
        if changed:
            bb.instructions = out
    return n_new


def _build_nc(has_qkvb, has_bo, has_mlpb):
    """has_qkvb: nonzero folded q/k/v biases (be1 != 0) -> general LN1.
    has_bo: nonzero bo. has_mlpb: nonzero b1/b2/be2 -> general LN2/MLP."""
    nc = bass.Bass("TRN2", target_bir_lowering=False, debug=False,
                   num_devices=N_CORES)
    fast1 = not has_qkvb
    fast2 = not has_mlpb

    P = nc.declare_dram_parameter
    xt_d = P("xt", [B_PER_CORE, C, T], F32R, isOutput=False)
    pdt = F32R if fast1 else BF16
    wq_d = P("wq", [2, 128, C], pdt, isOutput=False)
    wk_d = P("wk", [2, 128, C], pdt, isOutput=False)
    wv_d = P("wv", [2, 128, C], pdt, isOutput=False)
    wo_d = P("wo", [2, 128, C], BF16, isOutput=False)
    w1_d = P("w1", [2, 128, F], F32R if fast2 else BF16, isOutput=False)
    w2_d = P("w2", [8, 128, C], BF16, isOutput=False)
    maskt_d = P("maskt", [128, 128], BF16, isOutput=False)
    ident_d = P("ident", [128, 128], BF16, isOutput=False)
    id32_d = P("id32", [128, 128], F32, isOutput=False)
    onc_d = P("ones_c", [128, 128], F32R, isOutput=False)
    onv_d = P("ones_va", [128, C], BF16, isOutput=False)
    ont_d = P("ones_t", [1, T], BF16, isOutput=False)
    bq_d = P("bq", [128, 2], F32, isOutput=False)
    bk_d = P("bk", [128, 2], F32, isOutput=False)
    bv_d = P("bv", [128, C], F32, isOutput=False)
    bo_d = P("bo", [1, C], BF16, isOutput=False)
    b1_d = P("b1", [128, 8], F32, isOutput=False)
    b2_d = P("b2", [1, C], BF16, isOutput=False)
    yt_d = P("yt", [B_PER_CORE, C, T], F32, isOutput=True)

    with tile.TileContext(nc) as tc:
        with (
            tc.tile_pool(name="consts", bufs=1) as consts,
            tc.tile_pool(name="per_b", bufs=1) as per_b,
            tc.tile_pool(name="work", bufs=2) as work,
            tc.tile_pool(name="psum", bufs=2, space="PSUM") as psum,
        ):
            _kernel_body(
                nc, consts, per_b, work, psum,
                xt_d, wq_d, wk_d, wv_d, wo_d, w1_d, w2_d,
                maskt_d, ident_d, id32_d, onc_d, onv_d, ont_d,
                bq_d, bk_d, bv_d, bo_d, b1_d, b2_d, yt_d,
                fast1, fast2, has_qkvb, has_bo, has_mlpb,
            )
    _split_waits(nc)
    return nc


def _kernel_body(nc, consts, per_b, work, psum,
                 xt_d, wq_d, wk_d, wv_d, wo_d, w1_d, w2_d,
                 maskt_d, ident_d, id32_d, onc_d, onv_d, ont_d,
                 bq_d, bk_d, bv_d, bo_d, b1_d, b2_d, yt_d,
                 fast1, fast2, has_qkvb, has_bo, has_mlpb):
    NB = B_PER_CORE
    pdt = F32R if fast1 else BF16
    mdt = F32R if fast2 else BF16

    # ---- warm the ACT natural_log_exp table set while DMAs run ----
    eps_sb = consts.tile([128, 1], F32, tag="eps")
    nc.vector.memset(eps_sb, LN_EPS)
    warm_sb = consts.tile([128, 1], F32, tag="actwarm")
    nc.scalar.activation(out=warm_sb, in_=eps_sb, func=AF.Exp)
    nc.scalar.activation(out=warm_sb, in_=eps_sb, func=AF.Ln, bias=eps_sb,
                         scale=1.0)

    # ---- x first: everything downstream waits on it ----
    ones_stat = consts.tile([128, 128], F32R, tag="ones_stat")
    nc.sync.dma_start(out=ones_stat, in_=onc_d[:, :])
    xt = [[per_b.tile([128, T], F32R, tag=f"xt{b}_{ct}", name=f"xt{b}_{ct}")
           for ct in range(2)] for b in range(NB)]
    for b in range(NB):
        for ct in range(2):
            nc.sync.dma_start(out=xt[b][ct],
                              in_=xt_d[b, ct * 128:(ct + 1) * 128, :])

    # ---- constants ----
    wq_sb = [consts.tile([128, C], pdt, tag=f"wq{i}", name=f"wq{i}")
             for i in range(2)]
    wk_sb = [consts.tile([128, C], pdt, tag=f"wk{i}", name=f"wk{i}")
             for i in range(2)]
    wv_sb = [consts.tile([128, C], pdt, tag=f"wv{i}", name=f"wv{i}")
             for i in range(2)]
    wo_sb = [consts.tile([128, C], BF16, tag=f"wo{i}", name=f"wo{i}")
             for i in range(2)]
    w1_sb = [consts.tile([128, F], mdt, tag=f"w1{i}", name=f"w1{i}")
             for i in range(2)]
    w2_sb = [consts.tile([128, C], BF16, tag=f"w2{i}", name=f"w2{i}")
             for i in range(8)]
    for kt in range(2):
        nc.sync.dma_start(out=wq_sb[kt], in_=wq_d[kt])
        nc.sync.dma_start(out=wk_sb[kt], in_=wk_d[kt])
        nc.sync.dma_start(out=wv_sb[kt], in_=wv_d[kt])
        nc.sync.dma_start(out=wo_sb[kt], in_=wo_d[kt])
        nc.sync.dma_start(out=w1_sb[kt], in_=w1_d[kt])
    for kt in range(8):
        nc.sync.dma_start(out=w2_sb[kt], in_=w2_d[kt])

    maskt_sb = consts.tile([128, 128], BF16, tag="maskt")
    ident_sb = consts.tile([128, 128], BF16, tag="ident")
    ones_va = consts.tile([128, C], BF16, tag="ones_va")
    nc.sync.dma_start(out=maskt_sb, in_=maskt_d[:, :])
    nc.sync.dma_start(out=ident_sb, in_=ident_d[:, :])
    nc.sync.dma_start(out=ones_va, in_=onv_d[:, :])
    if fast1:
        id32_sb = consts.tile([128, 128], F32, tag="id32")
        nc.sync.dma_start(out=id32_sb, in_=id32_d[:, :])
    if has_qkvb:
        bq_sb = consts.tile([128, 2], F32, tag="bq")
        bk_sb = consts.tile([128, 2], F32, tag="bk")
        bv_sb = consts.tile([128, C], F32, tag="bv")
        nc.sync.dma_start(out=bq_sb, in_=bq_d[:, :])
        nc.sync.dma_start(out=bk_sb, in_=bk_d[:, :])
        nc.sync.dma_start(out=bv_sb, in_=bv_d[:, :])
    if has_mlpb:
        b1_sb = consts.tile([128, 8], F32, tag="b1")
        nc.sync.dma_start(out=b1_sb, in_=b1_d[:, :])
    if has_bo or has_mlpb:
        ones_row = consts.tile([1, T], BF16, tag="ones_row")
        nc.sync.dma_start(out=ones_row, in_=ont_d[:, :])
        bo_sb = consts.tile([1, C], BF16, tag="bo")
        nc.sync.dma_start(out=bo_sb, in_=bo_d[:, :])
        b2_sb = consts.tile([1, C], BF16, tag="b2")
        nc.sync.dma_start(out=b2_sb, in_=b2_d[:, :])

    # ---- per-batch persistent tiles ----
    q_sb = [[per_b.tile([128, T], BF16, tag=f"q{b}_{mt}", name=f"q{b}_{mt}")
             for mt in range(2)] for b in range(NB)]
    k_sb = [[per_b.tile([128, T], BF16, tag=f"k{b}_{mt}", name=f"k{b}_{mt}")
             for mt in range(2)] for b in range(NB)]
    vaug = [[per_b.tile([128, H, 128], BF16, tag=f"va{b}_{tt}",
                        name=f"va{b}_{tt}")
             for tt in range(6)] for b in range(NB)]
    ot = [[per_b.tile([128, T], BF16, tag=f"ot{b}_{mt}", name=f"ot{b}_{mt}")
           for mt in range(2)] for b in range(NB)]
    if not (fast1 and fast2):
        ht = [[per_b.tile([128, T], BF16, tag=f"ht{b}_{ct}",
                          name=f"ht{b}_{ct}") for ct in range(2)]
              for b in range(NB)]

    # ones halves of vaug: written once per run
    for b in range(NB):
        for tt in range(6):
            nc.vector.tensor_copy(
                out=vaug[b][tt][:, :, 64:128],
                in_=ones_va.rearrange("p (h d) -> p h d", h=H))

    def ln_stats(b, src, tag):
        """Partition-replicated LN stats for src (2x [128,T] f32r).
        Returns (alpha [128,T] f32 SBUF, ps_mu PSUM)."""
        sq = [work.tile([128, T], F32R, tag=f"ln_sq{ct}", bufs=2,
                        name=f"{tag}_sq{ct}") for ct in range(2)]
        for ct in range(2):
            nc.gpsimd.tensor_tensor(out=sq[ct], in0=src[ct], in1=src[ct],
                                    op=ALU.mult)
        ps_mu = psum.tile([128, T], F32, tag="pA", name=f"{tag}_mu")
        ps_ex2 = psum.tile([128, T], F32, tag="pB", name=f"{tag}_ex2")
        for ps, rhs in ((ps_mu, src), (ps_ex2, sq)):
            for kt in range(2):
                for st, ln in chunks(0, T, 512):
                    nc.tensor.matmul(
                        ps[:, st:st + ln], ones_stat, rhs[kt][:, st:st + ln],
                        start=(kt == 0), stop=(kt == 1))
        t2 = work.tile([128, T], F32, tag="ln_t2", bufs=2, name=f"{tag}_t2")
        alpha = work.tile([128, T], F32, tag="ln_al", bufs=4,
                          name=f"{tag}_al")
        nc.scalar.activation(out=t2, in_=ps_mu, func=AF.Square)
        nc.vector.tensor_tensor(out=t2, in0=ps_ex2, in1=t2, op=ALU.subtract)
        # alpha = (var+eps)^-0.5 = exp(-0.5*ln(var+eps)); Ln/Exp share the
        # natural_log_exp table set with the attention exp (no table swap).
        nc.scalar.activation(out=t2, in_=t2, func=AF.Ln, bias=eps_sb,
                             scale=1.0)
        nc.scalar.activation(out=alpha, in_=t2, func=AF.Exp, scale=-0.5)
        return alpha, ps_mu

    def ln_apply(b, src, alpha, ps_mu, out_tiles, tag):
        """General path: materialize h = (x - mu) * alpha into out_tiles."""
        beta = work.tile([128, T], F32, tag="ln_be", bufs=2,
                         name=f"{tag}_be")
        nc.vector.tensor_tensor(out=beta, in0=ps_mu, in1=alpha, op=ALU.mult)
        for ct in range(2):
            g1 = work.tile([128, T], F32, tag=f"ln_g{ct}", bufs=2,
                           name=f"{tag}_g{ct}")
            nc.gpsimd.tensor_tensor(out=g1, in0=src[ct], in1=alpha,
                                    op=ALU.mult)
            nc.gpsimd.tensor_tensor(out=out_tiles[ct], in0=g1, in1=beta,
                                    op=ALU.subtract)

    # ================= LN1 =================
    alphas = {}
    acol = {}
    for b in range(NB):
        alpha, ps_mu = ln_stats(b, xt[b], f"ln1_{b}")
        alphas[b] = alpha
        if not fast1:
            ln_apply(b, xt[b], alpha, ps_mu, ht[b], f"ln1_{b}")

    # ================= QKV =================
    for b in range(NB):
        alpha = alphas[b]
        mov = xt[b] if fast1 else ht[b]
        for name, w_sb, dst, bias_sb in (
                ("q", wq_sb, q_sb[b], bq_sb if has_qkvb else None),
                ("k", wk_sb, k_sb[b], bk_sb if has_qkvb else None)):
            for mt in range(2):
                ps = psum.tile([128, T], F32, tag="pA" if mt == 0 else "pB",
                               name=f"ps_{name}{b}_{mt}")
                for kt in range(2):
                    for st, ln in chunks(0, T, 512):
                        nc.tensor.matmul(
                            ps[:, st:st + ln],
                            w_sb[kt][:, mt * 128:(mt + 1) * 128],
                            mov[kt][:, st:st + ln],
                            start=(kt == 0), stop=(kt == 1))
                if fast1:
                    nc.vector.tensor_tensor(out=dst[mt], in0=ps, in1=alpha,
                                            op=ALU.mult)
                elif has_qkvb:
                    nc.scalar.activation(out=dst[mt], in_=ps,
                                         func=AF.Identity,
                                         bias=bias_sb[:, mt:mt + 1],
                                         scale=1.0)
                else:
                    nc.vector.tensor_copy(out=dst[mt], in_=ps)
        if fast1:
            # alpha columns (tokens on partitions) via PE transpose of the
            # replicated alpha tile; column 0 of each transposed block.
            ps_t = psum.tile([128, 6, 128], F32, tag="pA", name=f"ps_t{b}")
            for s in range(6):
                nc.tensor.transpose(ps_t[:, s, :],
                                    alpha[:, s * 128:(s + 1) * 128], id32_sb)
            ac = work.tile([128, 6], F32, tag="acol", bufs=4,
                           name=f"acol{b}")
            nc.vector.tensor_copy(out=ac, in_=ps_t[:, :, 0:1])
            acol[b] = ac
        for tt in range(6):
            ps = psum.tile([128, C], F32, tag="pA" if tt % 2 == 0 else "pB",
                           name=f"ps_v{b}_{tt}")
            for kt in range(2):
                nc.tensor.matmul(
                    ps, mov[kt][:, tt * 128:(tt + 1) * 128], wv_sb[kt],
                    start=(kt == 0), stop=(kt == 1))
            if fast1:
                nc.vector.tensor_scalar(
                    out=vaug[b][tt][:, :, 0:64],
                    in0=ps.rearrange("p (h d) -> p h d", h=H),
                    scalar1=acol[b][:, tt:tt + 1], scalar2=None,
                    op0=ALU.mult)
            elif has_qkvb:
                nc.vector.tensor_tensor(
                    out=vaug[b][tt][:, :, 0:64],
                    in0=ps.rearrange("p (h d) -> p h d", h=H),
                    in1=bv_sb.rearrange("p (h d) -> p h d", h=H),
                    op=ALU.add)
            else:
                nc.vector.tensor_copy(
                    out=vaug[b][tt][:, :, 0:64],
                    in_=ps.rearrange("p (h d) -> p h d", h=H))

    # ============ pipelined: ATTN / WO / (LN2+MLP) blocks ============
    def attn_block(b):
        # ---- attention ----
        for mt in range(2):
            po = [psum.tile([128, T], F32, tag="pB", name=f"po{b}_{mt}_{hh}")
                  for hh in range(2)]

            def emit_pv(unit):
                usi, uqlo, uw, upt = unit
                for hh in range(2):
                    nc.tensor.matmul(
                        po[hh][:, uqlo:uqlo + uw],
                        vaug[b][usi][:, 2 * mt + hh, :],
                        upt[:, hh, 0:uw],
                        start=(usi == 0), stop=(usi == 5))

            # PV trails scores/exp by one unit so the in-order PE stream
            # never waits on a fresh exp. Each unit gets its OWN pt tile:
            # deferring with a shared pt would read columns the next
            # unit's exp already overwrote.
            pending = None
            for si, qlo, w in ATTN_UNITS:
                diag = (qlo == si * 128)
                ps_s = psum.tile([128, 2, 512], F32, tag="pA",
                                 name=f"ps_s{b}_{mt}_{si}_{qlo}")
                for hh in range(2):
                    nc.tensor.matmul(
                        ps_s[:, hh, 0:w],
                        k_sb[b][mt][hh * 64:hh * 64 + 64,
                                    si * 128:si * 128 + 128],
                        q_sb[b][mt][hh * 64:hh * 64 + 64, qlo:qlo + w],
                        start=True, stop=not diag)
                if diag:
                    for hh in range(2):
                        nc.tensor.matmul(
                            ps_s[:, hh, 0:128], maskt_sb, ident_sb,
                            start=False, stop=True)
                pt = work.tile([128, 2, 512], BF16, tag="ptp", bufs=3,
                               name=f"pt{b}_{mt}_{si}_{qlo}")
                nc.scalar.activation(out=pt[:, :, 0:w],
                                     in_=ps_s[:, :, 0:w],
                                     func=AF.Exp, scale=HS ** -0.5)
                if pending is not None:
                    emit_pv(pending)
                pending = (si, qlo, w, pt)
            emit_pv(pending)
            rb = work.tile([64, 2, T], F32, tag="rb", bufs=2,
                           name=f"rb{b}_{mt}")
            for hh in range(2):
                nc.scalar.activation(out=rb[:, hh, :], in_=po[hh][64:128, :],
                                     func=AF.Ln)
            nc.scalar.activation(out=rb, in_=rb, func=AF.Exp, scale=-1.0)
            for hh in range(2):
                nc.vector.tensor_tensor(
                    out=ot[b][mt][hh * 64:hh * 64 + 64, :],
                    in0=po[hh][0:64, :], in1=rb[:, hh, :], op=ALU.mult)

    def wo_block(b):
        # ---- Wo + residual (in-place into xt) ----
        for mt in range(2):
            ps = psum.tile([128, T], F32, tag="pA", name=f"ps_r{b}_{mt}")
            for kt in range(2):
                for st, ln in chunks(0, T, 512):
                    nc.tensor.matmul(
                        ps[:, st:st + ln],
                        wo_sb[kt][:, mt * 128:(mt + 1) * 128],
                        ot[b][kt][:, st:st + ln],
                        start=(kt == 0), stop=(kt == 1) and not has_bo)
            if has_bo:
                for st, ln in chunks(0, T, 512):
                    nc.tensor.matmul(
                        ps[:, st:st + ln],
                        bo_sb[0:1, mt * 128:(mt + 1) * 128],
                        ones_row[:, st:st + ln], start=False, stop=True)
            nc.vector.tensor_tensor(out=xt[b][mt], in0=ps, in1=xt[b][mt],
                                    op=ALU.add)

    def post_block(b):
        # ---- LN2 ----
        alpha2, ps_mu2 = ln_stats(b, xt[b], f"ln2_{b}")
        if not fast2:
            ln_apply(b, xt[b], alpha2, ps_mu2, ht[b], f"ln2_{b}")

        # ---- MLP ----
        mov2 = xt[b] if fast2 else ht[b]
        ps_y = [psum.tile([128, T], F32, tag="pB", name=f"ps_y{b}_{mt}")
                for mt in range(2)]

        def emit_mlp2(f, ut):
            for mt in range(2):
                for st, ln in chunks(0, T, 512):
                    nc.tensor.matmul(
                        ps_y[mt][:, st:st + ln],
                        w2_sb[f][:, mt * 128:(mt + 1) * 128],
                        ut[:, st:st + ln],
                        start=(f == 0), stop=(f == 7) and not has_mlpb)

        pend = None  # MLP2 trails MLP1/relu by one f-block
        for f in range(8):
            ps_u = psum.tile([128, T], F32, tag="pA", name=f"ps_u{b}_{f}")
            for kt in range(2):
                for st, ln in chunks(0, T, 512):
                    nc.tensor.matmul(
                        ps_u[:, st:st + ln],
                        w1_sb[kt][:, f * 128:(f + 1) * 128],
                        mov2[kt][:, st:st + ln],
                        start=(kt == 0), stop=(kt == 1))
            ut = work.tile([128, T], BF16, tag="ut", bufs=4,
                           name=f"ut{b}_{f}")
            if has_mlpb:
                nc.scalar.activation(out=ut, in_=ps_u, func=AF.Relu,
                                     bias=b1_sb[:, f:f + 1], scale=1.0)
            elif f % 4 == 0:
                nc.scalar.activation(out=ut, in_=ps_u, func=AF.Relu)
            else:
                nc.vector.tensor_scalar_max(out=ut, in0=ps_u, scalar1=0.0)
            if pend is not None:
                emit_mlp2(*pend)
            pend = (f, ut)
        emit_mlp2(*pend)
        for mt in range(2):
            if has_mlpb:
                for st, ln in chunks(0, T, 512):
                    nc.tensor.matmul(
                        ps_y[mt][:, st:st + ln],
                        b2_sb[0:1, mt * 128:(mt + 1) * 128],
                        ones_row[:, st:st + ln], start=False, stop=True)
            if fast2:
                my = work.tile([128, T], F32, tag="my", bufs=2,
                               name=f"my{b}_{mt}")
                nc.vector.tensor_tensor(out=my, in0=ps_y[mt], in1=alpha2,
                                        op=ALU.mult)
                nc.vector.tensor_tensor(out=xt[b][mt], in0=my,
                                        in1=xt[b][mt], op=ALU.add)
            else:
                nc.vector.tensor_tensor(out=xt[b][mt], in0=ps_y[mt],
                                        in1=xt[b][mt], op=ALU.add)
            nc.sync.dma_start(out=yt_d[b, mt * 128:(mt + 1) * 128, :],
                              in_=xt[b][mt].bitcast(F32))

    # Emission order: an independent ATTN block sits between each WO and
    # its dependent LN2/MLP so the serial residual-add -> square -> stats
    # chain never starves the in-order PE stream.
    attn_block(0)
    attn_block(1)
    wo_block(0)
    attn_block(2)
    post_block(0)
    wo_block(1)
    attn_block(3)
    post_block(1)
    wo_block(2)
    wo_block(3)
    post_block(2)
    post_block(3)


_NC_CACHE = {}


def _prep_weights(Wq, Wk, Wv, Wo, bo, W1, b1, W2, b2, g1, be1, g2, be2,
                  fast1, fast2):
    f64 = np.float64
    g1, be1 = g1.astype(f64), be1.astype(f64)
    g2, be2 = g2.astype(f64), be2.astype(f64)
    CEN = np.eye(C) - np.ones((C, C)) / C  # centering projector

    def fold_qkv(W):  # W: [H, C, HS] -> folded [C, H*HS], bias [H*HS]
        Wraw = np.concatenate([W[h].astype(f64) for h in range(H)], axis=1)
        Wg = Wraw * g1[:, None]
        bias = be1 @ Wraw
        if fast1:
            Wg = CEN @ Wg
        return Wg, bias

    WqF, bq = fold_qkv(Wq)
    WkF, bk = fold_qkv(Wk)
    WvF, bv = fold_qkv(Wv)
    W1F = W1.astype(f64) * g2[:, None]
    b1F = b1.astype(f64) + be2 @ W1.astype(f64)
    if fast2:
        W1F = CEN @ W1F

    def f32(a):
        return np.ascontiguousarray(a, dtype=np.float32)

    def bf16(a):
        return np.ascontiguousarray(
            np.asarray(a, f64).astype(ml_dtypes.bfloat16))

    pcast = f32 if fast1 else bf16
    mcast = f32 if fast2 else bf16
    r = np.arange(128)
    maskt = np.where(r[None, :] <= r[:, None], 0.0, NEG_BIG)  # [k, m]

    return {
        "wq": pcast(WqF.reshape(2, 128, C)),
        "wk": pcast(WkF.reshape(2, 128, C)),
        "wv": pcast(WvF.reshape(2, 128, C)),
        "wo": bf16(np.asarray(Wo, f64).reshape(2, 128, C)),
        "w1": mcast(W1F.reshape(2, 128, F)),
        "w2": bf16(np.asarray(W2, f64).reshape(8, 128, C)),
        "bq": f32(bq.reshape(2, 128).T),
        "bk": f32(bk.reshape(2, 128).T),
        "bv": f32(np.broadcast_to(bv, (128, C))),
        "bo": bf16(np.asarray(bo, f64).reshape(1, C)),
        "b1": f32(b1F.reshape(8, 128).T),
        "b2": bf16(np.asarray(b2, f64).reshape(1, C)),
        "maskt": bf16(maskt),
        "ident": bf16(np.eye(128)),
        "id32": f32(np.eye(128)),
        "ones_c": f32(np.full((128, 128), 1.0 / C)),
        "ones_va": bf16(np.ones((128, C))),
        "ones_t": bf16(np.ones((1, T))),
    }


def kernel(x, Wq, Wk, Wv, Wo, bo, W1, b1, W2, b2, g1, be1, g2, be2,
           _trace=False):
    x = np.asarray(x, dtype=np.float32)
    has_qkvb = bool(np.any(np.asarray(be1)))
    has_bo = bool(np.any(np.asarray(bo)))
    has_mlpb = bool(np.any(np.asarray(b1)) or np.any(np.asarray(b2))
                    or np.any(np.asarray(be2)))
    flags = (has_qkvb, has_bo, has_mlpb)

    weights = _prep_weights(
        np.asarray(Wq), np.asarray(Wk), np.asarray(Wv), np.asarray(Wo),
        np.asarray(bo), np.asarray(W1), np.asarray(b1), np.asarray(W2),
        np.asarray(b2), np.asarray(g1), np.asarray(be1), np.asarray(g2),
        np.asarray(be2), not has_qkvb, not has_mlpb)

    if flags not in _NC_CACHE:
        _NC_CACHE[flags] = _build_nc(*flags)
    nc = _NC_CACHE[flags]

    xt = np.ascontiguousarray(x.transpose(0, 2, 1))  # [B, C, T]
    in_maps = []
    for core in range(N_CORES):
        m = dict(weights)
        m["xt"] = np.ascontiguousarray(
            xt[core * B_PER_CORE:(core + 1) * B_PER_CORE])
        in_maps.append(m)

    res = run_bass_kernel_spmd(nc, in_maps, list(range(N_CORES)),
                               trace=_trace)
    outs = [res.results[i]["yt"] for i in range(N_CORES)]  # [4, C, T] each
    y = np.concatenate(outs, axis=0).transpose(0, 2, 1)  # [B, T, C]
    if _trace:
        kernel.last_exec_time_ns = res.exec_time_ns
        kernel.last_results = res
    return np.ascontiguousarray(y)
